# revision 36
# baseline (speedup 1.0000x reference)
"""Self-attention kernel for Trainium2 (Bass/Tile), 8-core SPMD.

Problem: X [4, 4096, 512] f32
  S = X @ X^T per batch     [4, 4096, 4096]
  W = softmax(S, axis=-1)
  Y = W @ X                 [4, 4096, 512]

Sharding: data-parallel over batch (4 batches x 2 cores) + query-sequence
parallel within a batch (each core owns 2048 queries, sees all 4096 keys).
Host rolls each batch's key axis per core so the core's queries always sit
at rows/cols 0..2047 — the SPMD program is identical on all 8 cores and the
softmax reduction over keys is permutation-invariant.

Per-core device program (full attention, no shortcuts):
  - X^T (d-major, bf16 — score precision is softmax-insensitive) and
    X (n-major, float32r) resident in SBUF. f32r = fp32 streamed at bf16
    rate through the PE with 12-bit-mantissa operand rounding; PSUM
    accumulation is always full fp32. The P@X value matmul stays f32r so
    the output carries ~12-bit element accuracy.
  - per 128-query block: scores via PE (bf16), row-max on DVE over the
    bf16-rounded scores (so the top key's probability is exactly 1.0),
    exp on ACT (bf16 out, fused row-sum accumulation), 128x128 bf16 PE
    transposes of the probability block whose PSUM->SBUF copy converts
    to f32r on DVE, P^T @ X via PE (f32r), normalize by 1/l, DMA out.
"""

import ml_dtypes
import numpy as np

import concourse.bass as bass  # noqa: F401  (registers bass types)
import concourse.mybir as mybir
import concourse.tile as tile
from concourse import bacc
from concourse.bass_utils import run_bass_kernel_spmd
from concourse.masks import make_identity

F32 = mybir.dt.float32
F32R = mybir.dt.float32r
BF16 = mybir.dt.bfloat16
AX = mybir.AxisListType.X

P = 128          # partitions / query block
D = 512          # head dim
DC = D // P      # 4 d-chunks (contraction for scores)
NK = 4096        # keys per batch
NQ = 2048        # queries per core
NW = 512         # matmul moving width / PSUM bank width (fp32)
KT = NK // NW    # 8 key tiles per score row-block
KC = NK // P     # 32 key chunks (PV contraction)
NB = NQ // P     # 16 query blocks per core
N_CORES = 8
B = 4

_cached = None  # (nc, ...) build once per process


def _build_program():
    nc = bacc.Bacc("TRN2", target_bir_lowering=False, debug=False)
    xt_d = nc.dram_tensor("xt", [D, NK], BF16, kind="ExternalInput").ap()
    xn_d = nc.dram_tensor("xn", [NK, D], F32, kind="ExternalInput").ap()
    o_d = nc.dram_tensor("o", [NQ, D], F32, kind="ExternalOutput").ap()
    o_tiles = o_d.rearrange("(t p) d -> t p d", p=P)

    with tile.TileContext(nc) as tc:
        with tc.tile_pool(name="consts", bufs=1) as consts, \
             tc.tile_pool(name="pblk", bufs=4) as pblk, \
             tc.tile_pool(name="ptblk", bufs=2) as ptblk, \
             tc.tile_pool(name="stats", bufs=5) as stats, \
             tc.tile_pool(name="outp", bufs=1) as outp, \
             tc.tile_pool(name="ps_s", bufs=4, space="PSUM") as ps_s, \
             tc.tile_pool(name="ps_t", bufs=2, space="PSUM") as ps_t, \
             tc.tile_pool(name="ps_pv", bufs=2, space="PSUM") as ps_pv:

            xt_s = consts.tile([P, DC, NK], BF16)   # X^T (bf16), d on partitions
            xn_s = consts.tile([P, KC, D], F32R)    # X, keys on partitions

            # identity staging tile borrows a p_s slot (released on reuse)
            ident_f = pblk.tile([P, P], F32, name="ident_f", tag="p_s")
            make_identity(nc, ident_f)
            ident = consts.tile([P, P], BF16)
            nc.vector.tensor_copy(ident, ident_f)

            # Input DMA, first-needed-first on the SP HWDGE queue: the first
            # key tile's xt columns land as 4 small slivers (earliest PE
            # start), the rest of xt as one DMA per 512-key tile (so each
            # score tile's dependency releases as its slice arrives), then
            # xn in 16 groups (first needed by PV of block 0).
            xt_r = xt_d.rearrange("(c p) n -> p c n", p=P)
            for c in range(DC):
                nc.sync.dma_start(
                    xt_s[:, c, 0:NW],
                    xt_d[c * P:(c + 1) * P, 0:NW])
            for j in range(1, KT):
                nc.sync.dma_start(
                    xt_s[:, :, j * NW:(j + 1) * NW],
                    xt_r[:, :, j * NW:(j + 1) * NW])
            xn_r = xn_d.rearrange("(t p) d -> p t d", p=P)
            for g in range(16):
                nc.sync.dma_start(
                    xn_s[:, g * (KC // 16):(g + 1) * (KC // 16), :],
                    xn_r[:, g * (KC // 16):(g + 1) * (KC // 16), :].bitcast(F32R))

            def new_block():
                return {
                    "p_s": pblk.tile([P, KT, NW], BF16, name="p_s", tag="p_s"),
                    "mparts": stats.tile([P, KT], F32, name="mparts", tag="mparts"),
                    "negm": stats.tile([P, 1], F32, name="negm", tag="negm"),
                    "lparts": stats.tile([P, KT], F32, name="lparts", tag="lparts"),
                }

            def s_tile(qb, j, blk):
                """One 128x512 score tile: 4 accumulating MMs + copy + max.

                The PSUM->SBUF copy runs on ScalarE and the row-max on DVE;
                P^T copies live on DVE so they never queue behind exps in
                the ACT FIFO (engine streams are strict in-order)."""
                s_ps = ps_s.tile([P, NW], F32)
                for c in range(DC):
                    nc.tensor.matmul(
                        s_ps,
                        xt_s[:, c, qb * P:(qb + 1) * P],
                        xt_s[:, c, j * NW:(j + 1) * NW],
                        start=(c == 0), stop=(c == DC - 1))
                nc.scalar.copy(out=blk["p_s"][:, j, :], in_=s_ps)
                # max over the bf16-ROUNDED scores: the top key's exp argument
                # is then exactly 0, so its probability is exactly 1.0 in any
                # dtype and the l-normalization stays consistent.
                nc.vector.reduce_max(blk["mparts"][:, j:j + 1],
                                     blk["p_s"][:, j, :], axis=AX)

            def exp_block(blk):
                p_s, negm = blk["p_s"], blk["negm"]
                nc.vector.reduce_max(negm, blk["mparts"], axis=AX, negate=True)
                for j in range(KT):
                    nc.scalar.activation(
                        p_s[:, j, :], p_s[:, j, :],
                        mybir.ActivationFunctionType.Exp,
                        bias=negm, scale=1.0,
                        accum_out=blk["lparts"][:, j:j + 1])

            def s_phase(qb):
                """Scores + softmax numerator for query block qb."""
                blk = new_block()
                for j in range(KT):
                    s_tile(qb, j, blk)
                exp_block(blk)
                return blk

            def t_phase(blk):
                """Transpose the probability block into P^T layout."""
                p_s = blk["p_s"]
                pt_s = ptblk.tile([P, KC, P], F32R, name="pt_s", tag="pt_s")
                for g in range(KT):
                    t_ps = ps_t.tile([P, 4, P], BF16, name="t_ps", tag="t_ps")
                    for cc in range(4):
                        nc.tensor.transpose(
                            t_ps[:, cc, :],
                            p_s[:, g, cc * P:(cc + 1) * P],
                            ident)
                    nc.vector.tensor_copy(pt_s[:, 4 * g:4 * (g + 1), :], t_ps)
                blk["pt_s"] = pt_s

            def pv_start(blk):
                """First half of P^T @ X (keys 0..2047)."""
                pv_ps = ps_pv.tile([P, NW], F32, name="pv_ps", tag="pv_ps")
                blk["pv_ps"] = pv_ps
                for k in range(KC // 2):
                    nc.tensor.matmul(
                        pv_ps, blk["pt_s"][:, k, :], xn_s[:, k, :],
                        start=(k == 0), stop=False)

            def pv_finish(qb, blk):
                """Second half of P^T @ X, normalize by 1/l, store."""
                pt_s, lparts, pv_ps = blk["pt_s"], blk["lparts"], blk["pv_ps"]
                l_sum = stats.tile([P, 1], F32, name="l_sum", tag="l_sum")
                rl = stats.tile([P, 1], F32, name="rl", tag="rl")
                nc.vector.reduce_sum(l_sum, lparts, axis=AX)
                nc.vector.reciprocal(rl, l_sum)
                for k in range(KC // 2, KC):
                    nc.tensor.matmul(
                        pv_ps, pt_s[:, k, :], xn_s[:, k, :],
                        start=False, stop=(k == KC - 1))
                o_s = outp.tile([P, NW], F32, name="o_s", tag="o_s")
                nc.vector.tensor_scalar_mul(o_s, pv_ps, rl)
                nc.sync.dma_start(o_tiles[qb], o_s)

            def pv_phase(qb, blk):
                pv_start(blk)
                pv_finish(qb, blk)

            # Warmup: the first WARM blocks' score tiles interleave j-outer,
            # so the PE consumes each freshly-DMA'd xt sliver WARM times
            # while the next sliver streams in.
            WARM = 3
            warm_blks = [new_block() for _ in range(WARM)]
            for j in range(KT):
                for qb in range(WARM):
                    s_tile(qb, j, warm_blks[qb])
            for blk in warm_blks:
                exp_block(blk)

            # Steady emission: S_qb | T_{qb-3} | PV_{qb-4} — transposes
            # (which need no xn) fill the PE while the xn stream finishes.
            blks = {qb: warm_blks[qb] for qb in range(WARM)}
            for qb in range(WARM, NB):
                blks[qb] = s_phase(qb)
                t_phase(blks[qb - 3])
                if qb == 4:
                    pv_start(blks[0])          # first half rides the xn tail
                elif qb == 5:
                    pv_finish(0, blks[0])
                    pv_phase(1, blks[1])
                elif qb >= 6:
                    pv_phase(qb - 4, blks[qb - 4])
            for i in (NB - 3, NB - 2, NB - 1):
                t_phase(blks[i])
                pv_phase(i - 1, blks[i - 1])
            pv_phase(NB - 1, blks[NB - 1])

    nc.compile()
    return nc


def _get_program():
    global _cached
    if _cached is None:
        _cached = _build_program()
    return _cached


def _make_in_maps(X):
    in_maps = []
    for b in range(B):
        Xb = np.ascontiguousarray(X[b], dtype=np.float32)
        for h in range(2):
            qoff = h * NQ
            if qoff == 0:
                rolled = Xb
            else:
                rolled = np.ascontiguousarray(
                    np.concatenate([Xb[qoff:], Xb[:qoff]], axis=0))
            in_maps.append({
                "xn": rolled,
                "xt": np.ascontiguousarray(rolled.T).astype(ml_dtypes.bfloat16),
            })
    return in_maps


def run(X, trace=False, trace_kwargs=None):
    """Run the 8-core kernel on full X [4, 4096, 512]; returns (Y, results)."""
    X = np.asarray(X)
    assert X.shape == (B, NK, D), X.shape
    nc = _get_program()
    in_maps = _make_in_maps(X)
    res = run_bass_kernel_spmd(
        nc, in_maps, core_ids=list(range(N_CORES)),
        trace=trace, **(trace_kwargs or {}))
    out = np.empty((B, NK, D), dtype=np.float32)
    for b in range(B):
        for h in range(2):
            out[b, h * NQ:(h + 1) * NQ] = res.results[2 * b + h]["o"]
    return out, res


def kernel(X):
    out, _ = run(X)
    return out


# revision 37
# speedup vs baseline: 1.0393x; 1.0393x over previous
"""Self-attention kernel for Trainium2 (Bass/Tile), 8-core SPMD.

Problem: X [4, 4096, 512] f32
  S = X @ X^T per batch     [4, 4096, 4096]
  W = softmax(S, axis=-1)
  Y = W @ X                 [4, 4096, 512]

Sharding: data-parallel over batch (4 batches x 2 cores) + query-sequence
parallel within a batch (each core owns 2048 queries, sees all 4096 keys).
Host rolls each batch's key axis per core so the core's queries always sit
at rows/cols 0..2047 — the SPMD program is identical on all 8 cores and the
softmax reduction over keys is permutation-invariant.

Per-core device program (full attention, no shortcuts):
  - X^T (d-major, bf16 — score precision is softmax-insensitive) and
    X (n-major, float32r) resident in SBUF. f32r = fp32 streamed at bf16
    rate through the PE with 12-bit-mantissa operand rounding; PSUM
    accumulation is always full fp32. The P@X value matmul stays f32r so
    the output carries ~12-bit element accuracy.
  - per 128-query block: scores via PE (bf16), row-max on DVE over the
    bf16-rounded scores (so the top key's probability is exactly 1.0),
    exp on ACT (bf16 out, fused row-sum accumulation), 128x128 bf16 PE
    transposes of the probability block whose PSUM->SBUF copy converts
    to f32r on DVE, P^T @ X via PE (f32r), normalize by 1/l, DMA out.
"""

import ml_dtypes
import numpy as np

import concourse.bass as bass  # noqa: F401  (registers bass types)
import concourse.mybir as mybir
import concourse.tile as tile
from concourse import bacc
from concourse.bass_utils import run_bass_kernel_spmd
from concourse.masks import make_identity

F32 = mybir.dt.float32
F32R = mybir.dt.float32r
BF16 = mybir.dt.bfloat16
AX = mybir.AxisListType.X

P = 128          # partitions / query block
D = 512          # head dim
DC = D // P      # 4 d-chunks (contraction for scores)
NK = 4096        # keys per batch
NQ = 2048        # queries per core
NW = 512         # matmul moving width / PSUM bank width (fp32)
KT = NK // NW    # 8 key tiles per score row-block
KC = NK // P     # 32 key chunks (PV contraction)
NB = NQ // P     # 16 query blocks per core
N_CORES = 8
B = 4

_cached = None  # (nc, ...) build once per process


def _build_program():
    nc = bacc.Bacc("TRN2", target_bir_lowering=False, debug=False)
    xt_d = nc.dram_tensor("xt", [D, NK], BF16, kind="ExternalInput").ap()
    xn_d = nc.dram_tensor("xn", [NK, D], F32, kind="ExternalInput").ap()
    o_d = nc.dram_tensor("o", [NQ, D], F32, kind="ExternalOutput").ap()
    o_tiles = o_d.rearrange("(t p) d -> t p d", p=P)

    with tile.TileContext(nc) as tc:
        with tc.tile_pool(name="consts", bufs=1) as consts, \
             tc.tile_pool(name="pblk", bufs=4) as pblk, \
             tc.tile_pool(name="ptblk", bufs=2) as ptblk, \
             tc.tile_pool(name="stats", bufs=5) as stats, \
             tc.tile_pool(name="outp", bufs=1) as outp, \
             tc.tile_pool(name="ps_s", bufs=4, space="PSUM") as ps_s, \
             tc.tile_pool(name="ps_t", bufs=2, space="PSUM") as ps_t, \
             tc.tile_pool(name="ps_pv", bufs=2, space="PSUM") as ps_pv:

            xt_s = consts.tile([P, DC, NK], BF16)   # X^T (bf16), d on partitions
            xn_s = consts.tile([P, KC, D], F32R)    # X, keys on partitions
            # S = X X^T is symmetric: blocks 8..15 reuse the raw scores that
            # blocks 0..7 computed against keys 1024..2047 (transposed).
            rawsq = consts.tile([P, 8, 2, NW], BF16)

            # identity staging tile borrows a p_s slot (released on reuse)
            ident_f = pblk.tile([P, P], F32, name="ident_f", tag="p_s")
            make_identity(nc, ident_f)
            ident = consts.tile([P, P], BF16)
            nc.vector.tensor_copy(ident, ident_f)

            # Input DMA, first-needed-first on the SP HWDGE queue: the first
            # key tile's xt columns land as 4 small slivers (earliest PE
            # start), the rest of xt as one DMA per 512-key tile (so each
            # score tile's dependency releases as its slice arrives), then
            # xn in 16 groups (first needed by PV of block 0).
            xt_r = xt_d.rearrange("(c p) n -> p c n", p=P)
            for c in range(DC):
                nc.sync.dma_start(
                    xt_s[:, c, 0:NW],
                    xt_d[c * P:(c + 1) * P, 0:NW])
            for j in range(1, KT):
                nc.sync.dma_start(
                    xt_s[:, :, j * NW:(j + 1) * NW],
                    xt_r[:, :, j * NW:(j + 1) * NW])
            xn_r = xn_d.rearrange("(t p) d -> p t d", p=P)
            for g in range(16):
                nc.sync.dma_start(
                    xn_s[:, g * (KC // 16):(g + 1) * (KC // 16), :],
                    xn_r[:, g * (KC // 16):(g + 1) * (KC // 16), :].bitcast(F32R))

            def new_block():
                return {
                    "p_s": pblk.tile([P, KT, NW], BF16, name="p_s", tag="p_s"),
                    "mparts": stats.tile([P, KT], F32, name="mparts", tag="mparts"),
                    "negm": stats.tile([P, 1], F32, name="negm", tag="negm"),
                    "lparts": stats.tile([P, KT], F32, name="lparts", tag="lparts"),
                }

            def s_tile(qb, j, blk):
                """One 128x512 score tile: 4 accumulating MMs + copy + max.

                The PSUM->SBUF copy runs on ScalarE and the row-max on DVE;
                P^T copies live on DVE so they never queue behind exps in
                the ACT FIFO (engine streams are strict in-order)."""
                s_ps = ps_s.tile([P, NW], F32)
                for c in range(DC):
                    nc.tensor.matmul(
                        s_ps,
                        xt_s[:, c, qb * P:(qb + 1) * P],
                        xt_s[:, c, j * NW:(j + 1) * NW],
                        start=(c == 0), stop=(c == DC - 1))
                nc.scalar.copy(out=blk["p_s"][:, j, :], in_=s_ps)
                if qb < 8 and j in (2, 3):
                    # stash raw bf16 scores of the symmetric cross region
                    nc.vector.tensor_copy(rawsq[:, qb, j - 2, :], s_ps)
                # max over the bf16-ROUNDED scores: the top key's exp argument
                # is then exactly 0, so its probability is exactly 1.0 in any
                # dtype and the l-normalization stays consistent.
                nc.vector.reduce_max(blk["mparts"][:, j:j + 1],
                                     blk["p_s"][:, j, :], axis=AX)

            def exp_block(blk):
                p_s, negm = blk["p_s"], blk["negm"]
                nc.vector.reduce_max(negm, blk["mparts"], axis=AX, negate=True)
                for j in range(KT):
                    nc.scalar.activation(
                        p_s[:, j, :], p_s[:, j, :],
                        mybir.ActivationFunctionType.Exp,
                        bias=negm, scale=1.0,
                        accum_out=blk["lparts"][:, j:j + 1])

            def mirror_s_tile(qb, j, blk):
                """Score tile for keys 0..1023 of a block qb>=8: transpose
                the raw chunks block a computed against qb's key range."""
                m_ps = ps_s.tile([P, 4, P], BF16, name="m_ps", tag="s_ps")
                t_idx, cc_b = (qb - 8) // 4, (qb - 8) % 4
                for a in range(4 * j, 4 * j + 4):
                    nc.tensor.transpose(
                        m_ps[:, a % 4, :],
                        rawsq[:, a, t_idx, cc_b * P:(cc_b + 1) * P],
                        ident)
                nc.scalar.copy(out=blk["p_s"][:, j, :], in_=m_ps)
                nc.vector.reduce_max(blk["mparts"][:, j:j + 1],
                                     blk["p_s"][:, j, :], axis=AX)

            def s_phase(qb):
                """Scores + softmax numerator for query block qb."""
                blk = new_block()
                for j in range(KT):
                    if qb >= 8 and j < 2:
                        mirror_s_tile(qb, j, blk)
                    else:
                        s_tile(qb, j, blk)
                exp_block(blk)
                return blk

            def t_phase(blk):
                """Transpose the probability block into P^T layout."""
                p_s = blk["p_s"]
                pt_s = ptblk.tile([P, KC, P], F32R, name="pt_s", tag="pt_s")
                for g in range(KT):
                    t_ps = ps_t.tile([P, 4, P], BF16, name="t_ps", tag="t_ps")
                    for cc in range(4):
                        nc.tensor.transpose(
                            t_ps[:, cc, :],
                            p_s[:, g, cc * P:(cc + 1) * P],
                            ident)
                    nc.vector.tensor_copy(pt_s[:, 4 * g:4 * (g + 1), :], t_ps)
                blk["pt_s"] = pt_s

            def pv_start(blk):
                """First half of P^T @ X (keys 0..2047)."""
                pv_ps = ps_pv.tile([P, NW], F32, name="pv_ps", tag="pv_ps")
                blk["pv_ps"] = pv_ps
                for k in range(KC // 2):
                    nc.tensor.matmul(
                        pv_ps, blk["pt_s"][:, k, :], xn_s[:, k, :],
                        start=(k == 0), stop=False)

            def pv_finish(qb, blk):
                """Second half of P^T @ X, normalize by 1/l, store."""
                pt_s, lparts, pv_ps = blk["pt_s"], blk["lparts"], blk["pv_ps"]
                l_sum = stats.tile([P, 1], F32, name="l_sum", tag="l_sum")
                rl = stats.tile([P, 1], F32, name="rl", tag="rl")
                nc.vector.reduce_sum(l_sum, lparts, axis=AX)
                nc.vector.reciprocal(rl, l_sum)
                for k in range(KC // 2, KC):
                    nc.tensor.matmul(
                        pv_ps, pt_s[:, k, :], xn_s[:, k, :],
                        start=False, stop=(k == KC - 1))
                o_s = outp.tile([P, NW], F32, name="o_s", tag="o_s")
                nc.vector.tensor_scalar_mul(o_s, pv_ps, rl)
                nc.sync.dma_start(o_tiles[qb], o_s)

            def pv_phase(qb, blk):
                pv_start(blk)
                pv_finish(qb, blk)

            # Warmup: the first WARM blocks' score tiles interleave j-outer,
            # so the PE consumes each freshly-DMA'd xt sliver WARM times
            # while the next sliver streams in.
            WARM = 3
            warm_blks = [new_block() for _ in range(WARM)]
            for j in range(KT):
                for qb in range(WARM):
                    s_tile(qb, j, warm_blks[qb])
            for blk in warm_blks:
                exp_block(blk)

            # Steady emission: S_qb | T_{qb-3} | PV_{qb-4} — transposes
            # (which need no xn) fill the PE while the xn stream finishes.
            blks = {qb: warm_blks[qb] for qb in range(WARM)}
            for qb in range(WARM, NB):
                blks[qb] = s_phase(qb)
                t_phase(blks[qb - 3])
                if qb == 4:
                    pv_start(blks[0])          # first half rides the xn tail
                elif qb == 5:
                    pv_finish(0, blks[0])
                    pv_phase(1, blks[1])
                elif qb >= 6:
                    pv_phase(qb - 4, blks[qb - 4])
            for i in (NB - 3, NB - 2, NB - 1):
                t_phase(blks[i])
                pv_phase(i - 1, blks[i - 1])
            pv_phase(NB - 1, blks[NB - 1])

    nc.compile()
    return nc


def _get_program():
    global _cached
    if _cached is None:
        _cached = _build_program()
    return _cached


def _make_in_maps(X):
    in_maps = []
    for b in range(B):
        Xb = np.ascontiguousarray(X[b], dtype=np.float32)
        for h in range(2):
            qoff = h * NQ
            if qoff == 0:
                rolled = Xb
            else:
                rolled = np.ascontiguousarray(
                    np.concatenate([Xb[qoff:], Xb[:qoff]], axis=0))
            in_maps.append({
                "xn": rolled,
                "xt": np.ascontiguousarray(rolled.T).astype(ml_dtypes.bfloat16),
            })
    return in_maps


def run(X, trace=False, trace_kwargs=None):
    """Run the 8-core kernel on full X [4, 4096, 512]; returns (Y, results)."""
    X = np.asarray(X)
    assert X.shape == (B, NK, D), X.shape
    nc = _get_program()
    in_maps = _make_in_maps(X)
    res = run_bass_kernel_spmd(
        nc, in_maps, core_ids=list(range(N_CORES)),
        trace=trace, **(trace_kwargs or {}))
    out = np.empty((B, NK, D), dtype=np.float32)
    for b in range(B):
        for h in range(2):
            out[b, h * NQ:(h + 1) * NQ] = res.results[2 * b + h]["o"]
    return out, res


def kernel(X):
    out, _ = run(X)
    return out


# revision 38
# speedup vs baseline: 1.0624x; 1.0222x over previous
"""Self-attention kernel for Trainium2 (Bass/Tile), 8-core SPMD.

Problem: X [4, 4096, 512] f32
  S = X @ X^T per batch     [4, 4096, 4096]
  W = softmax(S, axis=-1)
  Y = W @ X                 [4, 4096, 512]

Sharding: data-parallel over batch (4 batches x 2 cores) + query-sequence
parallel within a batch (each core owns 2048 queries, sees all 4096 keys).
Host rolls each batch's key axis per core so the core's queries always sit
at rows/cols 0..2047 — the SPMD program is identical on all 8 cores and the
softmax reduction over keys is permutation-invariant.

Per-core device program (full attention, no shortcuts):
  - X^T (d-major, bf16 — score precision is softmax-insensitive) and
    X (n-major, float32r) resident in SBUF. f32r = fp32 streamed at bf16
    rate through the PE with 12-bit-mantissa operand rounding; PSUM
    accumulation is always full fp32. The P@X value matmul stays f32r so
    the output carries ~12-bit element accuracy.
  - per 128-query block: scores via PE (bf16), row-max on DVE over the
    bf16-rounded scores (so the top key's probability is exactly 1.0),
    exp on ACT (bf16 out, fused row-sum accumulation), 128x128 bf16 PE
    transposes of the probability block whose PSUM->SBUF copy converts
    to f32r on DVE, P^T @ X via PE (f32r), normalize by 1/l, DMA out.
"""

import ml_dtypes
import numpy as np

import concourse.bass as bass  # noqa: F401  (registers bass types)
import concourse.mybir as mybir
import concourse.tile as tile
from concourse import bacc
from concourse.bass_utils import run_bass_kernel_spmd
from concourse.masks import make_identity

F32 = mybir.dt.float32
F32R = mybir.dt.float32r
BF16 = mybir.dt.bfloat16
AX = mybir.AxisListType.X

P = 128          # partitions / query block
D = 512          # head dim
DC = D // P      # 4 d-chunks (contraction for scores)
NK = 4096        # keys per batch
NQ = 2048        # queries per core
NW = 512         # matmul moving width / PSUM bank width (fp32)
KT = NK // NW    # 8 key tiles per score row-block
KC = NK // P     # 32 key chunks (PV contraction)
NB = NQ // P     # 16 query blocks per core
N_CORES = 8
B = 4

_cached = None  # (nc, ...) build once per process


def _build_program():
    nc = bacc.Bacc("TRN2", target_bir_lowering=False, debug=False)
    xt_d = nc.dram_tensor("xt", [D, NK], BF16, kind="ExternalInput").ap()
    xn_d = nc.dram_tensor("xn", [NK, D], F32, kind="ExternalInput").ap()
    o_d = nc.dram_tensor("o", [NQ, D], F32, kind="ExternalOutput").ap()
    o_tiles = o_d.rearrange("(t p) d -> t p d", p=P)

    with tile.TileContext(nc) as tc:
        with tc.tile_pool(name="consts", bufs=1) as consts, \
             tc.tile_pool(name="pblk", bufs=4) as pblk, \
             tc.tile_pool(name="ptblk", bufs=2) as ptblk, \
             tc.tile_pool(name="stats", bufs=5) as stats, \
             tc.tile_pool(name="outp", bufs=1) as outp, \
             tc.tile_pool(name="ps_s", bufs=4, space="PSUM") as ps_s, \
             tc.tile_pool(name="ps_t", bufs=2, space="PSUM") as ps_t, \
             tc.tile_pool(name="ps_pv", bufs=2, space="PSUM") as ps_pv:

            xt_s = consts.tile([P, DC, NK], BF16)   # X^T (bf16), d on partitions
            xn_s = consts.tile([P, KC, D], F32R)    # X, keys on partitions
            # S = X X^T is symmetric: blocks 8..15 reuse the raw scores that
            # blocks 0..7 computed against keys 1024..2047 (transposed), and
            # within each 8-block pass the square's lower triangle reuses the
            # upper triangle's chunks (28 slots, shared by both passes since
            # pass A's triangle is fully consumed before pass B writes it).
            rawsq = consts.tile([P, 8, 2, NW], BF16)
            tri = consts.tile([P, 28, P], BF16)

            # identity staging tile borrows a p_s slot (released on reuse)
            ident_f = pblk.tile([P, P], F32, name="ident_f", tag="p_s")
            make_identity(nc, ident_f)
            ident = consts.tile([P, P], BF16)
            nc.vector.tensor_copy(ident, ident_f)

            # Input DMA, first-needed-first on the SP HWDGE queue: the first
            # key tile's xt columns land as 4 small slivers (earliest PE
            # start), the rest of xt as one DMA per 512-key tile (so each
            # score tile's dependency releases as its slice arrives), then
            # xn in 16 groups (first needed by PV of block 0).
            xt_r = xt_d.rearrange("(c p) n -> p c n", p=P)
            for c in range(DC):
                nc.sync.dma_start(
                    xt_s[:, c, 0:NW],
                    xt_d[c * P:(c + 1) * P, 0:NW])
            for j in range(1, KT):
                nc.sync.dma_start(
                    xt_s[:, :, j * NW:(j + 1) * NW],
                    xt_r[:, :, j * NW:(j + 1) * NW])
            xn_r = xn_d.rearrange("(t p) d -> p t d", p=P)
            for g in range(16):
                nc.sync.dma_start(
                    xn_s[:, g * (KC // 16):(g + 1) * (KC // 16), :],
                    xn_r[:, g * (KC // 16):(g + 1) * (KC // 16), :].bitcast(F32R))

            def new_block():
                return {
                    "p_s": pblk.tile([P, KT, NW], BF16, name="p_s", tag="p_s"),
                    "mparts": stats.tile([P, KT], F32, name="mparts", tag="mparts"),
                    "negm": stats.tile([P, 1], F32, name="negm", tag="negm"),
                    "lparts": stats.tile([P, KT], F32, name="lparts", tag="lparts"),
                }

            def s_tile(qb, j, blk):
                """One 128x512 score tile: 4 accumulating MMs + copy + max.

                The PSUM->SBUF copy runs on ScalarE and the row-max on DVE;
                P^T copies live on DVE so they never queue behind exps in
                the ACT FIFO (engine streams are strict in-order)."""
                s_ps = ps_s.tile([P, NW], F32)
                for c in range(DC):
                    nc.tensor.matmul(
                        s_ps,
                        xt_s[:, c, qb * P:(qb + 1) * P],
                        xt_s[:, c, j * NW:(j + 1) * NW],
                        start=(c == 0), stop=(c == DC - 1))
                nc.scalar.copy(out=blk["p_s"][:, j, :], in_=s_ps)
                if qb < 8 and j in (2, 3):
                    # stash raw bf16 scores of the symmetric cross region
                    nc.vector.tensor_copy(rawsq[:, qb, j - 2, :], s_ps)
                # max over the bf16-ROUNDED scores: the top key's exp argument
                # is then exactly 0, so its probability is exactly 1.0 in any
                # dtype and the l-normalization stays consistent.
                nc.vector.reduce_max(blk["mparts"][:, j:j + 1],
                                     blk["p_s"][:, j, :], axis=AX)

            def exp_block(blk):
                p_s, negm = blk["p_s"], blk["negm"]
                nc.vector.reduce_max(negm, blk["mparts"], axis=AX, negate=True)
                for j in range(KT):
                    nc.scalar.activation(
                        p_s[:, j, :], p_s[:, j, :],
                        mybir.ActivationFunctionType.Exp,
                        bias=negm, scale=1.0,
                        accum_out=blk["lparts"][:, j:j + 1])

            def tidx(a, b):
                # slot of chunk (a, b) with 0 <= a < b <= 7
                return a * 7 - a * (a - 1) // 2 + (b - a - 1)

            def square_s_tile(qb, j, blk):
                """In-pass symmetric tile: chunks below the diagonal are
                transposes of stored upper-triangle chunks; the rest is
                computed and the super-diagonal chunks stashed."""
                pa = qb // 8
                lb = qb - 8 * pa
                base_ka = 4 * (j - 2 * pa)
                m = min(max(lb - base_ka, 0), 4)
                if m > 0:
                    m_ps = ps_s.tile([P, 4, P], BF16, name="sq_ps", tag="s_ps")
                    for i in range(m):
                        nc.tensor.transpose(
                            m_ps[:, i, :],
                            tri[:, tidx(base_ka + i, lb), :], ident)
                    nc.scalar.copy(out=blk["p_s"][:, j, 0:m * P],
                                   in_=m_ps[:, 0:m, :])
                if m < 4:
                    s_ps = ps_s.tile([P, NW], F32, name="s_ps", tag="s_ps")
                    ncols = (4 - m) * P
                    for c in range(DC):
                        nc.tensor.matmul(
                            s_ps[:, 0:ncols],
                            xt_s[:, c, qb * P:(qb + 1) * P],
                            xt_s[:, c, j * NW + m * P:(j + 1) * NW],
                            start=(c == 0), stop=(c == DC - 1))
                    nc.scalar.copy(out=blk["p_s"][:, j, m * P:NW],
                                   in_=s_ps[:, 0:ncols])
                    for i in range(m, 4):
                        ka = base_ka + i
                        if ka > lb:
                            nc.vector.tensor_copy(
                                tri[:, tidx(lb, ka), :],
                                s_ps[:, (i - m) * P:(i - m + 1) * P])
                nc.vector.reduce_max(blk["mparts"][:, j:j + 1],
                                     blk["p_s"][:, j, :], axis=AX)

            def mirror_s_tile(qb, j, blk):
                """Score tile for keys 0..1023 of a block qb>=8: transpose
                the raw chunks block a computed against qb's key range."""
                m_ps = ps_s.tile([P, 4, P], BF16, name="m_ps", tag="s_ps")
                t_idx, cc_b = (qb - 8) // 4, (qb - 8) % 4
                for a in range(4 * j, 4 * j + 4):
                    nc.tensor.transpose(
                        m_ps[:, a % 4, :],
                        rawsq[:, a, t_idx, cc_b * P:(cc_b + 1) * P],
                        ident)
                nc.scalar.copy(out=blk["p_s"][:, j, :], in_=m_ps)
                nc.vector.reduce_max(blk["mparts"][:, j:j + 1],
                                     blk["p_s"][:, j, :], axis=AX)

            def s_phase(qb):
                """Scores + softmax numerator for query block qb."""
                blk = new_block()
                for j in range(KT):
                    emit_s_tile(qb, j, blk)
                exp_block(blk)
                return blk

            def t_phase(blk):
                """Transpose the probability block into P^T layout."""
                p_s = blk["p_s"]
                pt_s = ptblk.tile([P, KC, P], F32R, name="pt_s", tag="pt_s")
                for g in range(KT):
                    t_ps = ps_t.tile([P, 4, P], BF16, name="t_ps", tag="t_ps")
                    for cc in range(4):
                        nc.tensor.transpose(
                            t_ps[:, cc, :],
                            p_s[:, g, cc * P:(cc + 1) * P],
                            ident)
                    nc.vector.tensor_copy(pt_s[:, 4 * g:4 * (g + 1), :], t_ps)
                blk["pt_s"] = pt_s

            def pv_start(blk):
                """First half of P^T @ X (keys 0..2047)."""
                pv_ps = ps_pv.tile([P, NW], F32, name="pv_ps", tag="pv_ps")
                blk["pv_ps"] = pv_ps
                for k in range(KC // 2):
                    nc.tensor.matmul(
                        pv_ps, blk["pt_s"][:, k, :], xn_s[:, k, :],
                        start=(k == 0), stop=False)

            def pv_finish(qb, blk):
                """Second half of P^T @ X, normalize by 1/l, store."""
                pt_s, lparts, pv_ps = blk["pt_s"], blk["lparts"], blk["pv_ps"]
                l_sum = stats.tile([P, 1], F32, name="l_sum", tag="l_sum")
                rl = stats.tile([P, 1], F32, name="rl", tag="rl")
                nc.vector.reduce_sum(l_sum, lparts, axis=AX)
                nc.vector.reciprocal(rl, l_sum)
                for k in range(KC // 2, KC):
                    nc.tensor.matmul(
                        pv_ps, pt_s[:, k, :], xn_s[:, k, :],
                        start=False, stop=(k == KC - 1))
                o_s = outp.tile([P, NW], F32, name="o_s", tag="o_s")
                nc.vector.tensor_scalar_mul(o_s, pv_ps, rl)
                nc.sync.dma_start(o_tiles[qb], o_s)

            def pv_phase(qb, blk):
                pv_start(blk)
                pv_finish(qb, blk)

            def emit_s_tile(qb, j, blk):
                pa = qb // 8
                if pa == 1 and j < 2:
                    mirror_s_tile(qb, j, blk)       # cross-pass reuse
                elif 2 * pa <= j <= 2 * pa + 1:
                    square_s_tile(qb, j, blk)       # in-pass triangle
                else:
                    s_tile(qb, j, blk)

            # Warmup: the first WARM blocks' score tiles interleave j-outer,
            # so the PE consumes each freshly-DMA'd xt sliver WARM times
            # while the next sliver streams in.
            WARM = 3
            warm_blks = [new_block() for _ in range(WARM)]
            for j in range(KT):
                for qb in range(WARM):
                    emit_s_tile(qb, j, warm_blks[qb])
            for blk in warm_blks:
                exp_block(blk)

            # Steady emission: S_qb | T_{qb-3} | PV_{qb-4} — transposes
            # (which need no xn) fill the PE while the xn stream finishes.
            blks = {qb: warm_blks[qb] for qb in range(WARM)}
            for qb in range(WARM, NB):
                blks[qb] = s_phase(qb)
                t_phase(blks[qb - 3])
                if qb == 4:
                    pv_start(blks[0])          # first half rides the xn tail
                elif qb == 5:
                    pv_finish(0, blks[0])
                    pv_phase(1, blks[1])
                elif qb >= 6:
                    pv_phase(qb - 4, blks[qb - 4])
            for i in (NB - 3, NB - 2, NB - 1):
                t_phase(blks[i])
                pv_phase(i - 1, blks[i - 1])
            pv_phase(NB - 1, blks[NB - 1])

    nc.compile()
    return nc


def _get_program():
    global _cached
    if _cached is None:
        _cached = _build_program()
    return _cached


def _make_in_maps(X):
    in_maps = []
    for b in range(B):
        Xb = np.ascontiguousarray(X[b], dtype=np.float32)
        for h in range(2):
            qoff = h * NQ
            if qoff == 0:
                rolled = Xb
            else:
                rolled = np.ascontiguousarray(
                    np.concatenate([Xb[qoff:], Xb[:qoff]], axis=0))
            in_maps.append({
                "xn": rolled,
                "xt": np.ascontiguousarray(rolled.T).astype(ml_dtypes.bfloat16),
            })
    return in_maps


def run(X, trace=False, trace_kwargs=None):
    """Run the 8-core kernel on full X [4, 4096, 512]; returns (Y, results)."""
    X = np.asarray(X)
    assert X.shape == (B, NK, D), X.shape
    nc = _get_program()
    in_maps = _make_in_maps(X)
    res = run_bass_kernel_spmd(
        nc, in_maps, core_ids=list(range(N_CORES)),
        trace=trace, **(trace_kwargs or {}))
    out = np.empty((B, NK, D), dtype=np.float32)
    for b in range(B):
        for h in range(2):
            out[b, h * NQ:(h + 1) * NQ] = res.results[2 * b + h]["o"]
    return out, res


def kernel(X):
    out, _ = run(X)
    return out


# revision 39
# speedup vs baseline: 1.0628x; 1.0004x over previous
"""Self-attention kernel for Trainium2 (Bass/Tile), 8-core SPMD.

Problem: X [4, 4096, 512] f32
  S = X @ X^T per batch     [4, 4096, 4096]
  W = softmax(S, axis=-1)
  Y = W @ X                 [4, 4096, 512]

Sharding: data-parallel over batch (4 batches x 2 cores) + query-sequence
parallel within a batch (each core owns 2048 queries, sees all 4096 keys).
Host rolls each batch's key axis per core so the core's queries always sit
at rows/cols 0..2047 — the SPMD program is identical on all 8 cores and the
softmax reduction over keys is permutation-invariant.

Per-core device program (full attention, no shortcuts):
  - X^T (d-major, bf16 — score precision is softmax-insensitive) and
    X (n-major, float32r) resident in SBUF. f32r = fp32 streamed at bf16
    rate through the PE with 12-bit-mantissa operand rounding; PSUM
    accumulation is always full fp32. The P@X value matmul stays f32r so
    the output carries ~12-bit element accuracy.
  - per 128-query block: scores via PE (bf16), row-max on DVE over the
    bf16-rounded scores (so the top key's probability is exactly 1.0),
    exp on ACT (bf16 out, fused row-sum accumulation), 128x128 bf16 PE
    transposes of the probability block whose PSUM->SBUF copy converts
    to f32r on DVE, P^T @ X via PE (f32r), normalize by 1/l, DMA out.
"""

import ml_dtypes
import numpy as np

import concourse.bass as bass  # noqa: F401  (registers bass types)
import concourse.mybir as mybir
import concourse.tile as tile
from concourse import bacc
from concourse.bass_utils import run_bass_kernel_spmd
from concourse.masks import make_identity

F32 = mybir.dt.float32
F32R = mybir.dt.float32r
BF16 = mybir.dt.bfloat16
AX = mybir.AxisListType.X

P = 128          # partitions / query block
D = 512          # head dim
DC = D // P      # 4 d-chunks (contraction for scores)
NK = 4096        # keys per batch
NQ = 2048        # queries per core
NW = 512         # matmul moving width / PSUM bank width (fp32)
KT = NK // NW    # 8 key tiles per score row-block
KC = NK // P     # 32 key chunks (PV contraction)
NB = NQ // P     # 16 query blocks per core
N_CORES = 8
B = 4

_cached = None  # (nc, ...) build once per process


def _build_program():
    nc = bacc.Bacc("TRN2", target_bir_lowering=False, debug=False)
    xt_d = nc.dram_tensor("xt", [D, NK], BF16, kind="ExternalInput").ap()
    xn_d = nc.dram_tensor("xn", [NK, D], F32, kind="ExternalInput").ap()
    o_d = nc.dram_tensor("o", [NQ, D], F32, kind="ExternalOutput").ap()
    o_tiles = o_d.rearrange("(t p) d -> t p d", p=P)

    with tile.TileContext(nc) as tc:
        with tc.tile_pool(name="consts", bufs=1) as consts, \
             tc.tile_pool(name="pblk", bufs=4) as pblk, \
             tc.tile_pool(name="ptblk", bufs=2) as ptblk, \
             tc.tile_pool(name="stats", bufs=5) as stats, \
             tc.tile_pool(name="outp", bufs=1) as outp, \
             tc.tile_pool(name="ps_s", bufs=4, space="PSUM") as ps_s, \
             tc.tile_pool(name="ps_t", bufs=2, space="PSUM") as ps_t, \
             tc.tile_pool(name="ps_pv", bufs=2, space="PSUM") as ps_pv:

            xt_s = consts.tile([P, DC, NK], BF16)   # X^T (bf16), d on partitions
            xn_s = consts.tile([P, KC, D], F32R)    # X, keys on partitions
            # S = X X^T is symmetric: blocks 8..15 reuse the raw scores that
            # blocks 0..7 computed against keys 1024..2047 (transposed), and
            # within each 8-block pass the square's lower triangle reuses the
            # upper triangle's chunks (28 slots, shared by both passes since
            # pass A's triangle is fully consumed before pass B writes it).
            rawsq = consts.tile([P, 8, 2, NW], BF16)
            tri = consts.tile([P, 28, P], BF16)

            # identity staging tile borrows a p_s slot (released on reuse)
            ident_f = pblk.tile([P, P], F32, name="ident_f", tag="p_s")
            make_identity(nc, ident_f)
            ident = consts.tile([P, P], BF16)
            nc.vector.tensor_copy(ident, ident_f)

            # Input DMA, first-needed-first on the SP HWDGE queue: the first
            # key tile's xt columns land as 4 small slivers (earliest PE
            # start), the rest of xt as one DMA per 512-key tile (so each
            # score tile's dependency releases as its slice arrives), then
            # xn in 16 groups (first needed by PV of block 0).
            xt_r = xt_d.rearrange("(c p) n -> p c n", p=P)
            for c in range(DC):
                nc.sync.dma_start(
                    xt_s[:, c, 0:NW],
                    xt_d[c * P:(c + 1) * P, 0:NW])
            for j in range(1, KT):
                nc.sync.dma_start(
                    xt_s[:, :, j * NW:(j + 1) * NW],
                    xt_r[:, :, j * NW:(j + 1) * NW])
            xn_r = xn_d.rearrange("(t p) d -> p t d", p=P)
            for g in range(16):
                nc.sync.dma_start(
                    xn_s[:, g * (KC // 16):(g + 1) * (KC // 16), :],
                    xn_r[:, g * (KC // 16):(g + 1) * (KC // 16), :].bitcast(F32R))

            def new_block():
                return {
                    "p_s": pblk.tile([P, KT, NW], BF16, name="p_s", tag="p_s"),
                    "mparts": stats.tile([P, KT], F32, name="mparts", tag="mparts"),
                    "negm": stats.tile([P, 1], F32, name="negm", tag="negm"),
                    "lparts": stats.tile([P, KT], F32, name="lparts", tag="lparts"),
                }

            def s_tile(qb, j, blk):
                """One 128x512 score tile: 4 accumulating MMs + copy + max.

                The PSUM->SBUF copy runs on ScalarE and the row-max on DVE;
                P^T copies live on DVE so they never queue behind exps in
                the ACT FIFO (engine streams are strict in-order)."""
                s_ps = ps_s.tile([P, NW], F32)
                for c in range(DC):
                    nc.tensor.matmul(
                        s_ps,
                        xt_s[:, c, qb * P:(qb + 1) * P],
                        xt_s[:, c, j * NW:(j + 1) * NW],
                        start=(c == 0), stop=(c == DC - 1))
                nc.scalar.copy(out=blk["p_s"][:, j, :], in_=s_ps)
                if qb < 8 and j in (2, 3):
                    # stash raw bf16 scores of the symmetric cross region
                    nc.vector.tensor_copy(rawsq[:, qb, j - 2, :], s_ps)
                # max over the bf16-ROUNDED scores: the top key's exp argument
                # is then exactly 0, so its probability is exactly 1.0 in any
                # dtype and the l-normalization stays consistent.
                nc.vector.reduce_max(blk["mparts"][:, j:j + 1],
                                     blk["p_s"][:, j, :], axis=AX)

            def exp_block(blk):
                p_s, negm = blk["p_s"], blk["negm"]
                nc.vector.reduce_max(negm, blk["mparts"], axis=AX, negate=True)
                for j in range(KT):
                    nc.scalar.activation(
                        p_s[:, j, :], p_s[:, j, :],
                        mybir.ActivationFunctionType.Exp,
                        bias=negm, scale=1.0,
                        accum_out=blk["lparts"][:, j:j + 1])

            def tidx(a, b):
                # slot of chunk (a, b) with 0 <= a < b <= 7
                return a * 7 - a * (a - 1) // 2 + (b - a - 1)

            def square_s_tile(qb, j, blk):
                """In-pass symmetric tile: chunks below the diagonal are
                transposes of stored upper-triangle chunks; the rest is
                computed and the super-diagonal chunks stashed."""
                pa = qb // 8
                lb = qb - 8 * pa
                base_ka = 4 * (j - 2 * pa)
                m = min(max(lb - base_ka, 0), 4)
                # computed suffix first: its matmuls depend only on xt, so the
                # PE never waits on a just-stashed tri chunk for the mirrors
                if m < 4:
                    s_ps = ps_s.tile([P, NW], F32, name="s_ps", tag="s_ps")
                    ncols = (4 - m) * P
                    for c in range(DC):
                        nc.tensor.matmul(
                            s_ps[:, 0:ncols],
                            xt_s[:, c, qb * P:(qb + 1) * P],
                            xt_s[:, c, j * NW + m * P:(j + 1) * NW],
                            start=(c == 0), stop=(c == DC - 1))
                    nc.scalar.copy(out=blk["p_s"][:, j, m * P:NW],
                                   in_=s_ps[:, 0:ncols])
                    for i in range(m, 4):
                        ka = base_ka + i
                        if ka > lb:
                            nc.vector.tensor_copy(
                                tri[:, tidx(lb, ka), :],
                                s_ps[:, (i - m) * P:(i - m + 1) * P])
                if m > 0:
                    m_ps = ps_s.tile([P, 4, P], BF16, name="sq_ps", tag="s_ps")
                    for i in range(m):
                        nc.tensor.transpose(
                            m_ps[:, i, :],
                            tri[:, tidx(base_ka + i, lb), :], ident)
                    nc.scalar.copy(out=blk["p_s"][:, j, 0:m * P],
                                   in_=m_ps[:, 0:m, :])
                nc.vector.reduce_max(blk["mparts"][:, j:j + 1],
                                     blk["p_s"][:, j, :], axis=AX)

            def mirror_s_tile(qb, j, blk):
                """Score tile for keys 0..1023 of a block qb>=8: transpose
                the raw chunks block a computed against qb's key range."""
                m_ps = ps_s.tile([P, 4, P], BF16, name="m_ps", tag="s_ps")
                t_idx, cc_b = (qb - 8) // 4, (qb - 8) % 4
                for a in range(4 * j, 4 * j + 4):
                    nc.tensor.transpose(
                        m_ps[:, a % 4, :],
                        rawsq[:, a, t_idx, cc_b * P:(cc_b + 1) * P],
                        ident)
                nc.scalar.copy(out=blk["p_s"][:, j, :], in_=m_ps)
                nc.vector.reduce_max(blk["mparts"][:, j:j + 1],
                                     blk["p_s"][:, j, :], axis=AX)

            def s_phase(qb):
                """Scores + softmax numerator for query block qb."""
                blk = new_block()
                for j in range(KT):
                    emit_s_tile(qb, j, blk)
                exp_block(blk)
                return blk

            def t_phase(blk):
                """Transpose the probability block into P^T layout."""
                p_s = blk["p_s"]
                pt_s = ptblk.tile([P, KC, P], F32R, name="pt_s", tag="pt_s")
                for g in range(KT):
                    t_ps = ps_t.tile([P, 4, P], BF16, name="t_ps", tag="t_ps")
                    for cc in range(4):
                        nc.tensor.transpose(
                            t_ps[:, cc, :],
                            p_s[:, g, cc * P:(cc + 1) * P],
                            ident)
                    nc.vector.tensor_copy(pt_s[:, 4 * g:4 * (g + 1), :], t_ps)
                blk["pt_s"] = pt_s

            def pv_start(blk):
                """First half of P^T @ X (keys 0..2047)."""
                pv_ps = ps_pv.tile([P, NW], F32, name="pv_ps", tag="pv_ps")
                blk["pv_ps"] = pv_ps
                for k in range(KC // 2):
                    nc.tensor.matmul(
                        pv_ps, blk["pt_s"][:, k, :], xn_s[:, k, :],
                        start=(k == 0), stop=False)

            def pv_finish(qb, blk):
                """Second half of P^T @ X, normalize by 1/l, store."""
                pt_s, lparts, pv_ps = blk["pt_s"], blk["lparts"], blk["pv_ps"]
                l_sum = stats.tile([P, 1], F32, name="l_sum", tag="l_sum")
                rl = stats.tile([P, 1], F32, name="rl", tag="rl")
                nc.vector.reduce_sum(l_sum, lparts, axis=AX)
                nc.vector.reciprocal(rl, l_sum)
                for k in range(KC // 2, KC):
                    nc.tensor.matmul(
                        pv_ps, pt_s[:, k, :], xn_s[:, k, :],
                        start=False, stop=(k == KC - 1))
                o_s = outp.tile([P, NW], F32, name="o_s", tag="o_s")
                nc.vector.tensor_scalar_mul(o_s, pv_ps, rl)
                nc.sync.dma_start(o_tiles[qb], o_s)

            def pv_phase(qb, blk):
                pv_start(blk)
                pv_finish(qb, blk)

            def emit_s_tile(qb, j, blk):
                pa = qb // 8
                if pa == 1 and j < 2:
                    mirror_s_tile(qb, j, blk)       # cross-pass reuse
                elif 2 * pa <= j <= 2 * pa + 1:
                    square_s_tile(qb, j, blk)       # in-pass triangle
                else:
                    s_tile(qb, j, blk)

            # Warmup: the first WARM blocks' score tiles interleave j-outer,
            # so the PE consumes each freshly-DMA'd xt sliver WARM times
            # while the next sliver streams in.
            WARM = 3
            warm_blks = [new_block() for _ in range(WARM)]
            for j in range(KT):
                for qb in range(WARM):
                    emit_s_tile(qb, j, warm_blks[qb])
            for blk in warm_blks:
                exp_block(blk)

            # Steady emission: S_qb | T_{qb-3} | PV_{qb-4} — transposes
            # (which need no xn) fill the PE while the xn stream finishes.
            blks = {qb: warm_blks[qb] for qb in range(WARM)}
            for qb in range(WARM, NB):
                blks[qb] = s_phase(qb)
                t_phase(blks[qb - 3])
                if qb == 4:
                    pv_start(blks[0])          # first half rides the xn tail
                elif qb == 5:
                    pv_finish(0, blks[0])
                    pv_phase(1, blks[1])
                elif qb >= 6:
                    pv_phase(qb - 4, blks[qb - 4])
            for i in (NB - 3, NB - 2, NB - 1):
                t_phase(blks[i])
                pv_phase(i - 1, blks[i - 1])
            pv_phase(NB - 1, blks[NB - 1])

    nc.compile()
    return nc


def _get_program():
    global _cached
    if _cached is None:
        _cached = _build_program()
    return _cached


def _make_in_maps(X):
    in_maps = []
    for b in range(B):
        Xb = np.ascontiguousarray(X[b], dtype=np.float32)
        for h in range(2):
            qoff = h * NQ
            if qoff == 0:
                rolled = Xb
            else:
                rolled = np.ascontiguousarray(
                    np.concatenate([Xb[qoff:], Xb[:qoff]], axis=0))
            in_maps.append({
                "xn": rolled,
                "xt": np.ascontiguousarray(rolled.T).astype(ml_dtypes.bfloat16),
            })
    return in_maps


def run(X, trace=False, trace_kwargs=None):
    """Run the 8-core kernel on full X [4, 4096, 512]; returns (Y, results)."""
    X = np.asarray(X)
    assert X.shape == (B, NK, D), X.shape
    nc = _get_program()
    in_maps = _make_in_maps(X)
    res = run_bass_kernel_spmd(
        nc, in_maps, core_ids=list(range(N_CORES)),
        trace=trace, **(trace_kwargs or {}))
    out = np.empty((B, NK, D), dtype=np.float32)
    for b in range(B):
        for h in range(2):
            out[b, h * NQ:(h + 1) * NQ] = res.results[2 * b + h]["o"]
    return out, res


def kernel(X):
    out, _ = run(X)
    return out
